# revision 1
# baseline (speedup 1.0000x reference)
"""Trainium2 Bass kernel for CustomTemporalAttention.

B=8, T=1024, E=1024, H=16, D=64. Sharding: pure batch data-parallel across the
8 NeuronCores (core b computes batch element b end-to-end; weights and the tiny
bias table are replicated). No collectives.

Per-core math (torch Linear convention x @ W.T + b):
  qT = Wq @ query[b].T  (stored transposed: [E, T], channel-major)
  kT likewise; v in [T, E] layout augmented with a ones column per head.
  Per head h: S^T[tk, tq] = sum_d kT[d,tk] qT[d,tq]
  P^T = exp(0.125 * (S^T + 8*biasT)) via DVE add + ACT exp(scale=0.125)
  [num; den] = [v_h | 1].T @ P^T  accumulated over tk chunks (PSUM [65, 512])
  O^T_h = num / den ; y = O @ Wo.T + bo.

Matmul operands are bf16 (fp32 PSUM accumulate): full-chain numpy model gives
5.1e-3 max rel err vs the fp32 reference. bf16 restores fast-weight-load and
LDWEIGHTS/ MATMUL overlap that fp32/f32r modes forfeit.

Temporal bias: bias(q,k) = lerp(table[q - k + T-1]) with a global fractional
shift u = tanh(offset)/2.  blend[r] = a*tabp[r] + b*tabp[r+1] + c*tabp[r+2]
with a=relu(-u), b=1-|u|, c=relu(u) and tabp edge-padded — exact including the
clipped endpoints. Toeplitz tiles blend[C' - i + j] are materialized per head
as BSp[i, y] = rblend[y + i] (one overlapping-window DMA from DRAM scratch)
and read back with reversed free-dim slices (both patterns HW-verified).
"""

import sys

sys.path.insert(0, "/opt/trn_rl_repo")

import ml_dtypes
import numpy as np

import concourse.bass as bass
import concourse.mybir as mybir
import concourse.tile as tile
from concourse.bass_utils import run_bass_kernel_spmd

F32 = mybir.dt.float32
BF16 = mybir.dt.bfloat16
AF = mybir.ActivationFunctionType
ALU = mybir.AluOpType

B, T, E, H = 8, 1024, 1024, 16
D = E // H  # 64
TQ = 512
W_BSP = 1920


def _split_multi_waits(nc):
    """This walrus build accepts at most one sync-wait per instruction; hoist
    extras onto same-engine NoOp carriers placed immediately before."""
    n = 0
    for f in nc.m.functions:
        for blk in f.blocks:
            out = []
            for inst in blk.instructions:
                si = inst.sync_info
                waits = list(si.on_wait) if si and si.on_wait else []
                if len(waits) > 1:
                    for w in waits[:-1]:
                        n += 1
                        nop = mybir.InstNoOp(name=f"{inst.name}-ws{n}", ins=[], outs=[])
                        nop.engine = inst.engine
                        nop.sync_info = mybir.SyncInfo(on_wait=[w], on_update=[])
                        out.append(nop)
                    inst.sync_info = mybir.SyncInfo(
                        on_wait=[waits[-1]], on_update=list(si.on_update or [])
                    )
                out.append(inst)
            blk.instructions = out
    return n


def _craft(ap, dims, offset=None):
    c = ap.copy()
    c.ap = ap.ap.__class__(dims)
    if offset is not None:
        c.offset = offset
    return c


def _build():
    nc = bass.Bass()

    xqT = nc.declare_dram_parameter("xqT", [E, T], BF16, isOutput=False)
    xkT = nc.declare_dram_parameter("xkT", [E, T], BF16, isOutput=False)
    xvT = nc.declare_dram_parameter("xvT", [E, T], BF16, isOutput=False)
    wqT = nc.declare_dram_parameter("wqT", [E, E], BF16, isOutput=False)
    wkT = nc.declare_dram_parameter("wkT", [E, E], BF16, isOutput=False)
    wvT = nc.declare_dram_parameter("wvT", [E, E], BF16, isOutput=False)
    woT = nc.declare_dram_parameter("woT", [E, E], BF16, isOutput=False)
    bq2 = nc.declare_dram_parameter("bq2", [128, 8], F32, isOutput=False)
    bk2 = nc.declare_dram_parameter("bk2", [128, 8], F32, isOutput=False)
    bv1 = nc.declare_dram_parameter("bv1", [E], F32, isOutput=False)
    bo1 = nc.declare_dram_parameter("bo1", [E], F32, isOutput=False)
    rtabp = nc.declare_dram_parameter("rtabp", [H, 2 * T + 1], F32, isOutput=False)
    offs = nc.declare_dram_parameter("offs", [1], F32, isOutput=False)
    y_out = nc.declare_dram_parameter("y", [T, E], F32, isOutput=True)

    with tile.TileContext(nc) as tc:
        with (
            tc.tile_pool(name="persist", bufs=1) as persist,
            tc.tile_pool(name="small", bufs=1) as small,
            tc.tile_pool(name="dram", bufs=1, space="DRAM") as drp,
        ):
            qT = persist.tile([128, 8, T], BF16, tag="qT")
            kT = persist.tile([128, 8, T], BF16, tag="kT")
            vp = persist.tile([128, 8, H, D + 1], BF16, tag="vp")
            oT = persist.tile([128, 8, T], BF16, tag="oT")
            bvrep = persist.tile([128, E], F32, tag="bvrep")
            borep = persist.tile([128, E], F32, tag="borep")
            bqs = small.tile([128, 8], F32, tag="bqs")
            bks = small.tile([128, 8], F32, tag="bks")

            nc.sync.dma_start(out=bqs[:], in_=bq2[:])
            nc.sync.dma_start(out=bks[:], in_=bk2[:])
            nc.sync.dma_start(out=bvrep[:], in_=_craft(bv1[:], [[0, 128], [1, E]], 0))
            nc.sync.dma_start(out=borep[:], in_=_craft(bo1[:], [[0, 128], [1, E]], 0))

            # ---- phase 0: blended relative-position table ----
            p0ctx = tc.tile_pool(name="p0", bufs=1)
            p0 = p0ctx.__enter__()
            tab = p0.tile([H, 2 * T + 1], F32, tag="tab")
            nc.sync.dma_start(out=tab[:], in_=rtabp[:])
            off_sb = p0.tile([1, 1], F32, tag="off")
            nc.sync.dma_start(out=off_sb[:], in_=offs[None, :])
            th = p0.tile([1, 1], F32, tag="th")
            nc.scalar.activation(th[:], off_sb[:], AF.Tanh)
            w8 = p0.tile([1, 1], F32, tag="w8")
            nc.vector.tensor_scalar_mul(w8[:], th[:], 4.0)  # 8*u = 4*tanh
            abc = p0.tile([1, 3], F32, tag="abc")
            nc.vector.tensor_scalar(abc[:, 0:1], w8[:], -1.0, 0.0, ALU.mult, ALU.max)
            nc.vector.tensor_scalar(abc[:, 2:3], w8[:], 1.0, 0.0, ALU.mult, ALU.max)
            tsum = p0.tile([1, 1], F32, tag="tsum")
            nc.vector.tensor_tensor(tsum[:], abc[:, 0:1], abc[:, 2:3], ALU.add)
            nc.vector.tensor_scalar(abc[:, 1:2], tsum[:], -1.0, 8.0, ALU.mult, ALU.add)
            abc_dram = drp.tile([3], F32, tag="abc_dram")
            nc.gpsimd.dma_start(out=abc_dram[None, :], in_=abc[:])
            abc16 = p0.tile([H, 3], F32, tag="abc16")
            nc.gpsimd.dma_start(out=abc16[:], in_=_craft(abc_dram[:], [[0, H], [1, 3]], 0))

            nblend = 2 * T - 1
            rb = p0.tile([H, nblend], F32, tag="rb")
            rb_t = p0.tile([H, nblend], F32, tag="rb_t")
            nc.vector.tensor_scalar(rb[:], tab[:, 2 : 2 + nblend], abc16[:, 0:1], None, ALU.mult)
            nc.vector.tensor_scalar(rb_t[:], tab[:, 1 : 1 + nblend], abc16[:, 1:2], None, ALU.mult)
            nc.vector.tensor_tensor(rb[:], rb[:], rb_t[:], ALU.add)
            nc.vector.tensor_scalar(rb_t[:], tab[:, 0:nblend], abc16[:, 2:3], None, ALU.mult)
            nc.vector.tensor_tensor(rb[:], rb[:], rb_t[:], ALU.add)
            # erb = exp(bias) of the blended table (bf16): per-head exp(b)
            # slabs are then pure overlapping-window DMA loads, no ACT work
            erb = p0.tile([H, nblend], BF16, tag="erb")
            nc.scalar.activation(erb[:], rb[:], AF.Exp, scale=0.125)
            erb_dram = drp.tile([H, nblend], BF16, tag="erb_dram")
            nc.gpsimd.dma_start(out=erb_dram[:], in_=erb[:])
            p0ctx.__exit__(None, None, None)

            # ---- phase 1: projections ----
            with (
                tc.tile_pool(name="xt", bufs=2) as xtp,
                tc.tile_pool(name="wt", bufs=10) as wtp,
                tc.tile_pool(name="wtv", bufs=1) as wtvp,
                tc.tile_pool(name="pps", bufs=4, space="PSUM") as pps,
            ):
                for name, x_in, w_in, dst, bias_sb in (
                    ("q", xqT, wqT, qT, bqs),
                    ("k", xkT, wkT, kT, bks),
                ):
                    xt = []
                    for eo in range(8):
                        for tq in range(2):
                            t_ = xtp.tile([128, TQ], BF16, tag=f"xt{eo}_{tq}")
                            nc.sync.dma_start(
                                out=t_[:],
                                in_=x_in[128 * eo : 128 * eo + 128, TQ * tq : TQ * tq + TQ],
                            )
                            xt.append(t_)
                    for fo in range(8):
                        ps = [pps.tile([128, TQ], F32, tag="pps", name=f"pp{fo}_{i}") for i in range(2)]
                        for eo in range(8):
                            wt_ = wtp.tile([128, 128], BF16, tag="wt")
                            nc.sync.dma_start(
                                out=wt_[:],
                                in_=w_in[128 * eo : 128 * eo + 128, 128 * fo : 128 * fo + 128],
                            )
                            for tq in range(2):
                                nc.tensor.matmul(
                                    ps[tq][:],
                                    wt_[:],
                                    xt[2 * eo + tq][:],
                                    start=(eo == 0),
                                    stop=(eo == 7),
                                )
                        for tq in range(2):
                            nc.vector.tensor_scalar(
                                dst[:, fo, TQ * tq : TQ * tq + TQ],
                                ps[tq][:],
                                1.0,
                                bias_sb[:, fo : fo + 1],
                                ALU.mult,
                                ALU.add,
                            )

                xt = []
                for eo in range(8):
                    for to2 in range(2):
                        t_ = xtp.tile([128, TQ], BF16, tag=f"xt{eo}_{to2}")
                        nc.sync.dma_start(
                            out=t_[:],
                            in_=xvT[128 * eo : 128 * eo + 128, TQ * to2 : TQ * to2 + TQ],
                        )
                        xt.append(t_)
                for fv in range(2):
                    wts = []
                    for eo in range(8):
                        wt_ = wtvp.tile([128, TQ], BF16, tag=f"wtv{eo}")
                        nc.sync.dma_start(
                            out=wt_[:],
                            in_=wvT[128 * eo : 128 * eo + 128, TQ * fv : TQ * fv + TQ],
                        )
                        wts.append(wt_)
                    for to in range(8):
                        to2, toi = divmod(to, 4)
                        ps = pps.tile([128, TQ], F32, tag="pps")
                        for eo in range(8):
                            nc.tensor.matmul(
                                ps[:],
                                xt[2 * eo + to2][:, 128 * toi : 128 * toi + 128],
                                wts[eo][:],
                                start=(eo == 0),
                                stop=(eo == 7),
                            )
                        nc.vector.tensor_tensor(
                            vp[:, to, 8 * fv : 8 * fv + 8, 0:D],
                            ps[:].rearrange("p (h d) -> p h d", d=D),
                            bvrep[:, TQ * fv : TQ * fv + TQ].rearrange(
                                "p (h d) -> p h d", d=D
                            ),
                            ALU.add,
                        )
                nc.vector.memset(vp[:, :, :, D : D + 1], 1.0)

            # ---- phase 2: attention ----
            # exp(0.125*(S + 8b)) = exp(0.125*S) * exp(b): the Toeplitz bias is
            # applied multiplicatively with a per-head exp(b) slab (bf16, DVE
            # 4x mode) instead of an fp32 PSUM add, and PV matmuls are emitted
            # as a block after the S block so the PE stream never stalls on
            # the exp chain.
            with (
                tc.tile_pool(name="eb", bufs=2) as ebp,
                tc.tile_pool(name="pt", bufs=4) as ptp,
                tc.tile_pool(name="pt0", bufs=3) as pt0p,
                tc.tile_pool(name="sm", bufs=6) as smp,
                tc.tile_pool(name="onum", bufs=6) as onp,
                tc.tile_pool(name="sps", bufs=2, space="PSUM") as sps,
                tc.tile_pool(name="ops", bufs=4, space="PSUM") as ops,
                tc.tile_pool(name="dr2", bufs=6, space="DRAM") as drp2,
            ):
                ebs = {}
                pend = {}

                def emit_eb(hh):
                    eb_ = ebp.tile([128, W_BSP], BF16, tag="eb", name=f"eb{hh}")
                    nc.sync.dma_start(
                        out=eb_[:],
                        in_=_craft(erb_dram[:], [[1, 128], [1, W_BSP]], hh * nblend),
                    )
                    ebs[hh] = eb_

                def _norm_stage_a(hh):
                    st = pend[hh]
                    for tq in range(2):
                        opsum_ = st["opsum"][tq]
                        den = smp.tile([1, TQ], F32, tag="den", name=f"den{hh}_{tq}")
                        nc.vector.tensor_copy(out=den[:], in_=opsum_[D : D + 1, :])
                        onum = onp.tile([64, TQ], F32, tag="onum", name=f"on{hh}_{tq}")
                        nc.vector.tensor_copy(out=onum[:], in_=opsum_[0:D, :])
                        den_dram = drp2.tile([TQ], F32, tag="dend", name=f"dd{hh}_{tq}")
                        nc.gpsimd.dma_start(out=den_dram[None, :], in_=den[:])
                        den4 = smp.tile([128, 4], F32, tag="den4", name=f"d4{hh}_{tq}")
                        nc.gpsimd.dma_start(
                            out=den4[:], in_=den_dram.rearrange("(f p) -> p f", p=128)
                        )
                        st["den"].append(den)
                        st["onum"].append(onum)
                        st["den4"].append(den4)

                def _norm_stage_b(hh):
                    st = pend[hh]
                    for tq in range(2):
                        rec4 = smp.tile([128, 4], F32, tag="rec4", name=f"r4{hh}_{tq}")
                        nc.vector.reciprocal(rec4[:], st["den4"][tq][:])
                        rec_dram = drp2.tile([TQ], F32, tag="recd", name=f"rd{hh}_{tq}")
                        nc.gpsimd.dma_start(
                            out=rec_dram.rearrange("(f p) -> p f", p=128), in_=rec4[:]
                        )
                        rep = onp.tile([64, TQ], F32, tag="rep", name=f"rp{hh}_{tq}")
                        nc.gpsimd.dma_start(
                            out=rep[:], in_=_craft(rec_dram[:], [[0, 64], [1, TQ]], 0)
                        )
                        st["rec4"].append(rec4)
                        st["rep"].append(rep)

                def _norm_stage_c(hh):
                    st = pend.pop(hh)
                    po_, hp0_ = st["po"], st["hp0"]
                    for tq in range(2):
                        if hp0_ == 0:
                            nc.gpsimd.tensor_tensor(
                                oT[0:64, po_, TQ * tq : TQ * tq + TQ],
                                st["onum"][tq][:],
                                st["rep"][tq][:],
                                ALU.mult,
                            )
                        else:
                            onrm = onp.tile([64, TQ], BF16, tag="onrm", name=f"om{hh}_{tq}")
                            nc.gpsimd.tensor_tensor(
                                onrm[:], st["onum"][tq][:], st["rep"][tq][:], ALU.mult
                            )
                            nc.gpsimd.dma_start(
                                out=oT[64:128, po_, TQ * tq : TQ * tq + TQ], in_=onrm[:]
                            )

                emit_eb(0)
                for h in range(H):
                    hp0 = 64 * (h % 2)
                    po = h // 2
                    if h + 1 < H:
                        emit_eb(h + 1)
                    eb = ebs.pop(h)
                    opsum = [
                        ops.tile([D + 1, TQ], F32, tag="ops", name=f"op{h}_{i}")
                        for i in range(2)
                    ]
                    pts = {}

                    def emit_pv(cc):
                        pt_ = pts.pop(cc)
                        for tq in range(2):
                            nc.tensor.matmul(
                                opsum[tq][:],
                                vp[:, cc, h, :],
                                pt_[:, TQ * tq : TQ * tq + TQ],
                                start=(cc == 0),
                                stop=(cc == 7),
                            )

                    for c in range(8):
                        # both tq halves share one 2-bank PSUM tile so the exp
                        # and bias-multiply run as single [128,1024] ops
                        spsum = sps.tile([128, 2 * TQ], F32, tag="sps", name=f"sp{h}_{c}")
                        for tq in range(2):
                            nc.tensor.matmul(
                                spsum[:, TQ * tq : TQ * tq + TQ],
                                kT[hp0 : hp0 + 64, po, 128 * c : 128 * c + 128],
                                qT[hp0 : hp0 + 64, po, TQ * tq : TQ * tq + TQ],
                                start=True,
                                stop=True,
                            )
                        pt0 = pt0p.tile([128, 2 * TQ], BF16, tag="pt0")
                        nc.scalar.activation(pt0[:], spsum[:], AF.Exp, scale=0.125)
                        s0 = 1023 + 128 * c
                        pt = ptp.tile([128, 2 * TQ], BF16, tag="pt")
                        nc.vector.tensor_tensor(
                            pt[:],
                            pt0[:],
                            eb[:, s0 - (2 * TQ - 1) : s0 + 1][:, ::-1],
                            ALU.mult,
                        )
                        pts[c] = pt
                        if c >= 1:
                            emit_pv(c - 1)
                    emit_pv(7)
                    # normalize runs 1-3 heads deferred so nothing in any
                    # engine FIFO waits on a fresh PV-block or DMA roundtrip
                    pend[h] = {"po": po, "hp0": hp0, "opsum": opsum, "den": [],
                               "onum": [], "den4": [], "rec4": [], "rep": []}
                    if h - 1 in pend:
                        _norm_stage_a(h - 1)
                    if h - 2 in pend:
                        _norm_stage_b(h - 2)
                    if h - 3 in pend:
                        _norm_stage_c(h - 3)
                _norm_stage_a(H - 1)
                _norm_stage_b(H - 2)
                _norm_stage_c(H - 3)
                _norm_stage_b(H - 1)
                _norm_stage_c(H - 2)
                _norm_stage_c(H - 1)

            # ---- phase 3: output projection ----
            with (
                tc.tile_pool(name="wo", bufs=1) as wop,
                tc.tile_pool(name="yst", bufs=4) as ystp,
                tc.tile_pool(name="pps3", bufs=4, space="PSUM") as pps3,
            ):
                for fo2 in range(2):
                    wts = []
                    for co in range(8):
                        wt_ = wop.tile([128, TQ], BF16, tag=f"wo{co}")
                        nc.sync.dma_start(
                            out=wt_[:],
                            in_=woT[128 * co : 128 * co + 128, TQ * fo2 : TQ * fo2 + TQ],
                        )
                        wts.append(wt_)
                    for to in range(8):
                        ps = pps3.tile([128, TQ], F32, tag="pps3")
                        for co in range(8):
                            nc.tensor.matmul(
                                ps[:],
                                oT[:, co, 128 * to : 128 * to + 128],
                                wts[co][:],
                                start=(co == 0),
                                stop=(co == 7),
                            )
                        yst = ystp.tile([128, TQ], F32, tag="yst")
                        nc.vector.tensor_tensor(
                            yst[:], ps[:], borep[:, TQ * fo2 : TQ * fo2 + TQ], ALU.add
                        )
                        nc.sync.dma_start(
                            out=y_out[128 * to : 128 * to + 128, TQ * fo2 : TQ * fo2 + TQ],
                            in_=yst[:],
                        )

    _split_multi_waits(nc)
    return nc


_NC_CACHE = None


def _get_nc():
    global _NC_CACHE
    if _NC_CACHE is None:
        _NC_CACHE = _build()
    return _NC_CACHE


def _bf(x):
    return np.ascontiguousarray(np.asarray(x, np.float32).astype(ml_dtypes.bfloat16))


def _prepare_in_maps(
    query, key_, value, Wq, bq, Wk, bk, Wv, bv, Wo, bo, bias_table, offset
):
    query = np.asarray(query, np.float32)
    key_ = np.asarray(key_, np.float32)
    value = np.asarray(value, np.float32)
    shared = {
        "wqT": _bf(np.asarray(Wq, np.float32).T),
        "wkT": _bf(np.asarray(Wk, np.float32).T),
        "wvT": _bf(np.asarray(Wv, np.float32).T),
        "woT": _bf(np.asarray(Wo, np.float32).T),
        "bq2": np.ascontiguousarray(np.asarray(bq, np.float32).reshape(8, 128).T),
        "bk2": np.ascontiguousarray(np.asarray(bk, np.float32).reshape(8, 128).T),
        "bv1": np.ascontiguousarray(np.asarray(bv, np.float32)),
        "bo1": np.ascontiguousarray(np.asarray(bo, np.float32)),
        "offs": np.ascontiguousarray(np.asarray(offset, np.float32)),
    }
    tab = np.asarray(bias_table, np.float32)  # [2T-1, H]
    pad = np.concatenate([tab[0:1], tab, tab[-1:]], axis=0)  # [2T+1, H]
    shared["rtabp"] = np.ascontiguousarray(pad[::-1].T)  # [H, 2T+1]

    in_maps = []
    for b in range(B):
        m = dict(shared)
        m["xqT"] = _bf(query[b].T)
        m["xkT"] = _bf(key_[b].T)
        m["xvT"] = _bf(value[b].T)
        in_maps.append(m)
    return in_maps


def kernel(**inputs):
    in_maps = _prepare_in_maps(
        inputs["query"], inputs["key_"], inputs["value"],
        inputs["Wq"], inputs["bq"], inputs["Wk"], inputs["bk"],
        inputs["Wv"], inputs["bv"], inputs["Wo"], inputs["bo"],
        inputs["bias_table"], inputs["offset"],
    )
    nc = _get_nc()
    res = run_bass_kernel_spmd(nc, in_maps, list(range(B)))
    out = np.stack([res.results[b]["y"] for b in range(B)], axis=0)
    return out.astype(np.float32)



# revision 9
# speedup vs baseline: 1.2656x; 1.2656x over previous
"""Trainium2 Bass kernel for CustomTemporalAttention.

B=8, T=1024, E=1024, H=16, D=64. Sharding: pure batch data-parallel across the
8 NeuronCores (core b computes batch element b end-to-end; weights and the tiny
bias table are replicated). No collectives.

Per-core math (torch Linear convention x @ W.T + b):
  qT = Wq @ query[b].T  (stored transposed: [E, T], channel-major)
  kT likewise; v in [T, E] layout augmented with a ones column per head.
  Per head h: S^T[tk, tq] = sum_d kT[d,tk] qT[d,tq]
  P^T = exp(0.125 * (S^T + 8*biasT)) via DVE add + ACT exp(scale=0.125)
  [num; den] = [v_h | 1].T @ P^T  accumulated over tk chunks (PSUM [65, 512])
  O^T_h = num / den ; y = O @ Wo.T + bo.

Matmul operands are bf16 (fp32 PSUM accumulate): full-chain numpy model gives
5.1e-3 max rel err vs the fp32 reference. bf16 restores fast-weight-load and
LDWEIGHTS/ MATMUL overlap that fp32/f32r modes forfeit.

Temporal bias: bias(q,k) = lerp(table[q - k + T-1]) with a global fractional
shift u = tanh(offset)/2.  blend[r] = a*tabp[r] + b*tabp[r+1] + c*tabp[r+2]
with a=relu(-u), b=1-|u|, c=relu(u) and tabp edge-padded — exact including the
clipped endpoints. Toeplitz tiles blend[C' - i + j] are materialized per head
as BSp[i, y] = rblend[y + i] (one overlapping-window DMA from DRAM scratch)
and read back with reversed free-dim slices (both patterns HW-verified).
"""

import sys

sys.path.insert(0, "/opt/trn_rl_repo")

import ml_dtypes
import numpy as np

import concourse.bass as bass
import concourse.mybir as mybir
import concourse.tile as tile
from concourse.bass_utils import run_bass_kernel_spmd

F32 = mybir.dt.float32
F32R = mybir.dt.float32r
BF16 = mybir.dt.bfloat16
AF = mybir.ActivationFunctionType
ALU = mybir.AluOpType

B, T, E, H = 8, 1024, 1024, 16
D = E // H  # 64
TQ = 512
W_BSP = 1920


def _split_multi_waits(nc):
    """This walrus build accepts at most one sync-wait per instruction; hoist
    extras onto same-engine NoOp carriers placed immediately before."""
    n = 0
    for f in nc.m.functions:
        for blk in f.blocks:
            out = []
            for inst in blk.instructions:
                si = inst.sync_info
                waits = list(si.on_wait) if si and si.on_wait else []
                if len(waits) > 1:
                    for w in waits[:-1]:
                        n += 1
                        nop = mybir.InstNoOp(name=f"{inst.name}-ws{n}", ins=[], outs=[])
                        nop.engine = inst.engine
                        nop.sync_info = mybir.SyncInfo(on_wait=[w], on_update=[])
                        out.append(nop)
                    inst.sync_info = mybir.SyncInfo(
                        on_wait=[waits[-1]], on_update=list(si.on_update or [])
                    )
                out.append(inst)
            blk.instructions = out
    return n


def _craft(ap, dims, offset=None):
    c = ap.copy()
    c.ap = ap.ap.__class__(dims)
    if offset is not None:
        c.offset = offset
    return c


def _build():
    nc = bass.Bass()

    xqT = nc.declare_dram_parameter("xqT", [E, T], BF16, isOutput=False)
    xkT = nc.declare_dram_parameter("xkT", [E, T], BF16, isOutput=False)
    xvT = nc.declare_dram_parameter("xvT", [E, T], BF16, isOutput=False)
    wqT = nc.declare_dram_parameter("wqT", [E, E], BF16, isOutput=False)
    wkT = nc.declare_dram_parameter("wkT", [E, E], BF16, isOutput=False)
    wvT = nc.declare_dram_parameter("wvT", [E, E], BF16, isOutput=False)
    woT = nc.declare_dram_parameter("woT", [E, E], BF16, isOutput=False)
    bq2 = nc.declare_dram_parameter("bq2", [128, 8], F32, isOutput=False)
    bk2 = nc.declare_dram_parameter("bk2", [128, 8], F32, isOutput=False)
    bv1 = nc.declare_dram_parameter("bv1", [E], F32, isOutput=False)
    bo1 = nc.declare_dram_parameter("bo1", [E], F32, isOutput=False)
    rtabp = nc.declare_dram_parameter("rtabp", [H, 2 * T + 1], F32, isOutput=False)
    offs = nc.declare_dram_parameter("offs", [1], F32, isOutput=False)
    y_out = nc.declare_dram_parameter("y", [T, E], F32, isOutput=True)

    with tile.TileContext(nc) as tc:
        with (
            tc.tile_pool(name="persist", bufs=1) as persist,
            tc.tile_pool(name="small", bufs=1) as small,
            tc.tile_pool(name="dram", bufs=1, space="DRAM") as drp,
        ):
            qT = persist.tile([128, 8, T], BF16, tag="qT")
            kT = persist.tile([128, 8, T], BF16, tag="kT")
            vp = persist.tile([128, 8, H, D + 1], BF16, tag="vp")
            oT = persist.tile([128, 8, T], BF16, tag="oT")
            bvrep = persist.tile([128, E], F32, tag="bvrep")
            borep = persist.tile([128, E], F32, tag="borep")
            bqs = small.tile([128, 8], F32, tag="bqs")
            bks = small.tile([128, 8], F32, tag="bks")

            nc.sync.dma_start(out=bqs[:], in_=bq2[:])
            nc.sync.dma_start(out=bks[:], in_=bk2[:])
            nc.sync.dma_start(out=bvrep[:], in_=_craft(bv1[:], [[0, 128], [1, E]], 0))
            nc.sync.dma_start(out=borep[:], in_=_craft(bo1[:], [[0, 128], [1, E]], 0))

            # ---- phase 0: blended relative-position table ----
            p0ctx = tc.tile_pool(name="p0", bufs=1)
            p0 = p0ctx.__enter__()
            tab = p0.tile([H, 2 * T + 1], F32, tag="tab")
            nc.sync.dma_start(out=tab[:], in_=rtabp[:])
            off_sb = p0.tile([1, 1], F32, tag="off")
            nc.sync.dma_start(out=off_sb[:], in_=offs[None, :])
            th = p0.tile([1, 1], F32, tag="th")
            nc.scalar.activation(th[:], off_sb[:], AF.Tanh)
            w8 = p0.tile([1, 1], F32, tag="w8")
            nc.vector.tensor_scalar_mul(w8[:], th[:], 4.0)  # 8*u = 4*tanh
            abc = p0.tile([1, 3], F32, tag="abc")
            nc.vector.tensor_scalar(abc[:, 0:1], w8[:], -1.0, 0.0, ALU.mult, ALU.max)
            nc.vector.tensor_scalar(abc[:, 2:3], w8[:], 1.0, 0.0, ALU.mult, ALU.max)
            tsum = p0.tile([1, 1], F32, tag="tsum")
            nc.vector.tensor_tensor(tsum[:], abc[:, 0:1], abc[:, 2:3], ALU.add)
            nc.vector.tensor_scalar(abc[:, 1:2], tsum[:], -1.0, 8.0, ALU.mult, ALU.add)
            abc_dram = drp.tile([3], F32, tag="abc_dram")
            nc.gpsimd.dma_start(out=abc_dram[None, :], in_=abc[:])
            abc16 = p0.tile([H, 3], F32, tag="abc16")
            nc.gpsimd.dma_start(out=abc16[:], in_=_craft(abc_dram[:], [[0, H], [1, 3]], 0))

            nblend = 2 * T - 1
            rb = p0.tile([H, nblend], F32, tag="rb")
            rb_t = p0.tile([H, nblend], F32, tag="rb_t")
            nc.vector.tensor_scalar(rb[:], tab[:, 2 : 2 + nblend], abc16[:, 0:1], None, ALU.mult)
            nc.vector.tensor_scalar(rb_t[:], tab[:, 1 : 1 + nblend], abc16[:, 1:2], None, ALU.mult)
            nc.vector.tensor_tensor(rb[:], rb[:], rb_t[:], ALU.add)
            nc.vector.tensor_scalar(rb_t[:], tab[:, 0:nblend], abc16[:, 2:3], None, ALU.mult)
            nc.vector.tensor_tensor(rb[:], rb[:], rb_t[:], ALU.add)
            # erb = exp(bias) of the blended table (bf16): per-head exp(b)
            # slabs are then pure overlapping-window DMA loads, no ACT work
            erb = p0.tile([H, nblend], BF16, tag="erb")
            nc.scalar.activation(erb[:], rb[:], AF.Exp, scale=0.125)
            erb_dram = drp.tile([H, nblend], BF16, tag="erb_dram")
            nc.gpsimd.dma_start(out=erb_dram[:], in_=erb[:])
            p0ctx.__exit__(None, None, None)

            # ---- phase 1: projections ----
            with (
                tc.tile_pool(name="xt", bufs=2) as xtp,
                tc.tile_pool(name="wt", bufs=10) as wtp,
                tc.tile_pool(name="wtv", bufs=1) as wtvp,
                tc.tile_pool(name="pps", bufs=4, space="PSUM") as pps,
            ):
                for name, x_in, w_in, dst, bias_sb in (
                    ("q", xqT, wqT, qT, bqs),
                    ("k", xkT, wkT, kT, bks),
                ):
                    xt = []
                    for eo in range(8):
                        for tq in range(2):
                            t_ = xtp.tile([128, TQ], BF16, tag=f"xt{eo}_{tq}")
                            nc.sync.dma_start(
                                out=t_[:],
                                in_=x_in[128 * eo : 128 * eo + 128, TQ * tq : TQ * tq + TQ],
                            )
                            xt.append(t_)
                    for fo in range(8):
                        ps = [pps.tile([128, TQ], F32, tag="pps", name=f"pp{fo}_{i}") for i in range(2)]
                        for eo in range(8):
                            wt_ = wtp.tile([128, 128], BF16, tag="wt")
                            nc.sync.dma_start(
                                out=wt_[:],
                                in_=w_in[128 * eo : 128 * eo + 128, 128 * fo : 128 * fo + 128],
                            )
                            for tq in range(2):
                                nc.tensor.matmul(
                                    ps[tq][:],
                                    wt_[:],
                                    xt[2 * eo + tq][:],
                                    start=(eo == 0),
                                    stop=(eo == 7),
                                )
                        for tq in range(2):
                            nc.vector.tensor_scalar(
                                dst[:, fo, TQ * tq : TQ * tq + TQ],
                                ps[tq][:],
                                1.0,
                                bias_sb[:, fo : fo + 1],
                                ALU.mult,
                                ALU.add,
                            )

                xt = []
                for eo in range(8):
                    for to2 in range(2):
                        t_ = xtp.tile([128, TQ], BF16, tag=f"xt{eo}_{to2}")
                        nc.sync.dma_start(
                            out=t_[:],
                            in_=xvT[128 * eo : 128 * eo + 128, TQ * to2 : TQ * to2 + TQ],
                        )
                        xt.append(t_)
                for fv in range(2):
                    wts = []
                    for eo in range(8):
                        wt_ = wtvp.tile([128, TQ], BF16, tag=f"wtv{eo}")
                        nc.sync.dma_start(
                            out=wt_[:],
                            in_=wvT[128 * eo : 128 * eo + 128, TQ * fv : TQ * fv + TQ],
                        )
                        wts.append(wt_)
                    for to in range(8):
                        to2, toi = divmod(to, 4)
                        ps = pps.tile([128, TQ], F32, tag="pps")
                        for eo in range(8):
                            nc.tensor.matmul(
                                ps[:],
                                xt[2 * eo + to2][:, 128 * toi : 128 * toi + 128],
                                wts[eo][:],
                                start=(eo == 0),
                                stop=(eo == 7),
                            )
                        nc.vector.tensor_tensor(
                            vp[:, to, 8 * fv : 8 * fv + 8, 0:D],
                            ps[:].rearrange("p (h d) -> p h d", d=D),
                            bvrep[:, TQ * fv : TQ * fv + TQ].rearrange(
                                "p (h d) -> p h d", d=D
                            ),
                            ALU.add,
                        )
                nc.vector.memset(vp[:, :, :, D : D + 1], 1.0)

            # ---- phase 2: attention ----
            # exp(0.125*(S + 8b)) = exp(0.125*S) * exp(b): the Toeplitz bias is
            # applied multiplicatively with a per-head exp(b) slab (bf16, DVE
            # 4x mode) instead of an fp32 PSUM add, and PV matmuls are emitted
            # as a block after the S block so the PE stream never stalls on
            # the exp chain.
            #
            # softmax normalize is fully on-chip: DVE reciprocal of the den row
            # (PSUM partition 64) -> rank-1 f32r matmul ones[1,64] (x) rec
            # broadcasts it across 64 PSUM partitions -> one DVE multiply
            # (PSUM x PSUM -> SBUF bf16). No DRAM roundtrips.
            with (
                tc.tile_pool(name="eb", bufs=2) as ebp,
                tc.tile_pool(name="pt", bufs=4) as ptp,
                tc.tile_pool(name="pt0", bufs=3) as pt0p,
                tc.tile_pool(name="sm", bufs=4) as smp,
                tc.tile_pool(name="onum", bufs=2) as onp,
                tc.tile_pool(name="sps", bufs=2, space="PSUM") as sps,
                tc.tile_pool(name="ops", bufs=3, space="PSUM") as ops,
                tc.tile_pool(name="rps", bufs=1, space="PSUM") as rps,
            ):
                ones1f = smp.tile([1, D], F32, tag="ones1f")
                nc.vector.memset(ones1f[:], 1.0)
                ones1 = smp.tile([1, D], F32R, tag="ones1")
                with nc.allow_low_precision(reason="f32r == f32 bits"):
                    nc.vector.tensor_copy(out=ones1[:], in_=ones1f[:])
                ebs = {}
                pend = {}

                def emit_eb(hh):
                    eb_ = ebp.tile([128, W_BSP], BF16, tag="eb", name=f"eb{hh}")
                    nc.sync.dma_start(
                        out=eb_[:],
                        in_=_craft(erb_dram[:], [[1, 128], [1, W_BSP]], hh * nblend),
                    )
                    ebs[hh] = eb_

                def _norm_recip(hh):
                    st = pend[hh]
                    for tq in range(2):
                        rec = smp.tile([1, TQ], F32R, tag="rec", name=f"rc{hh}_{tq}")
                        with nc.allow_low_precision(reason="f32r == f32 bits"):
                            nc.vector.reciprocal(rec[:], st["opsum"][tq][D : D + 1, :])
                        st["rec"].append(rec)

                def _norm_tail(hh, tq):
                    st = pend[hh]
                    po_, hp0_ = st["po"], st["hp0"]
                    recb = rps.tile([D, TQ], F32, tag="recb", name=f"rb{hh}_{tq}")
                    nc.tensor.matmul(
                        recb[:],
                        ones1[:],
                        st["rec"][tq][:],
                        start=True,
                        stop=True,
                    )
                    recs = smp.tile([D, TQ], F32, tag="recs", name=f"rs{hh}_{tq}")
                    nc.scalar.activation(recs[:], recb[:], AF.Copy)
                    if hp0_ == 0:
                        nc.vector.tensor_tensor(
                            oT[0:64, po_, TQ * tq : TQ * tq + TQ],
                            st["opsum"][tq][0:D, :],
                            recs[:],
                            ALU.mult,
                        )
                    else:
                        onrm = onp.tile([64, TQ], BF16, tag="onrm", name=f"om{hh}_{tq}")
                        nc.vector.tensor_tensor(
                            onrm[:], st["opsum"][tq][0:D, :], recs[:], ALU.mult
                        )
                        nc.gpsimd.dma_start(
                            out=oT[64:128, po_, TQ * tq : TQ * tq + TQ], in_=onrm[:]
                        )

                emit_eb(0)
                for h in range(H):
                    hp0 = 64 * (h % 2)
                    po = h // 2
                    if h + 1 < H:
                        emit_eb(h + 1)
                    eb = ebs.pop(h)
                    if h - 1 in pend:
                        _norm_recip(h - 1)
                    opsum = [
                        ops.tile([D + 1, TQ], F32, tag="ops", name=f"op{h}_{i}")
                        for i in range(2)
                    ]
                    pts = {}

                    def emit_pv(cc):
                        pt_ = pts.pop(cc)
                        for tq in range(2):
                            nc.tensor.matmul(
                                opsum[tq][:],
                                vp[:, cc, h, :],
                                pt_[:, TQ * tq : TQ * tq + TQ],
                                start=(cc == 0),
                                stop=(cc == 7),
                            )

                    for c in range(8):
                        if h - 1 in pend and c in (1, 2):
                            _norm_tail(h - 1, c - 1)
                            if c == 2:
                                pend.pop(h - 1)
                        # both tq halves share one 2-bank PSUM tile so the exp
                        # and bias-multiply run as single [128,1024] ops
                        spsum = sps.tile([128, 2 * TQ], F32, tag="sps", name=f"sp{h}_{c}")
                        for tq in range(2):
                            nc.tensor.matmul(
                                spsum[:, TQ * tq : TQ * tq + TQ],
                                kT[hp0 : hp0 + 64, po, 128 * c : 128 * c + 128],
                                qT[hp0 : hp0 + 64, po, TQ * tq : TQ * tq + TQ],
                                start=True,
                                stop=True,
                            )
                        pt0 = pt0p.tile([128, 2 * TQ], BF16, tag="pt0")
                        nc.scalar.activation(pt0[:], spsum[:], AF.Exp, scale=0.125)
                        s0 = 1023 + 128 * c
                        pt = ptp.tile([128, 2 * TQ], BF16, tag="pt")
                        nc.vector.tensor_tensor(
                            pt[:],
                            pt0[:],
                            eb[:, s0 - (2 * TQ - 1) : s0 + 1][:, ::-1],
                            ALU.mult,
                        )
                        pts[c] = pt
                        if c >= 1:
                            emit_pv(c - 1)
                    emit_pv(7)
                    pend[h] = {"po": po, "hp0": hp0, "opsum": opsum, "rec": []}
                _norm_recip(H - 1)
                _norm_tail(H - 1, 0)
                _norm_tail(H - 1, 1)
                pend.pop(H - 1)

            # ---- phase 3: output projection ----
            with (
                tc.tile_pool(name="wo", bufs=1) as wop,
                tc.tile_pool(name="yst", bufs=4) as ystp,
                tc.tile_pool(name="pps3", bufs=4, space="PSUM") as pps3,
            ):
                for fo2 in range(2):
                    wts = []
                    for co in range(8):
                        wt_ = wop.tile([128, TQ], BF16, tag=f"wo{co}")
                        nc.sync.dma_start(
                            out=wt_[:],
                            in_=woT[128 * co : 128 * co + 128, TQ * fo2 : TQ * fo2 + TQ],
                        )
                        wts.append(wt_)
                    for to in range(8):
                        ps = pps3.tile([128, TQ], F32, tag="pps3")
                        for co in range(8):
                            nc.tensor.matmul(
                                ps[:],
                                oT[:, co, 128 * to : 128 * to + 128],
                                wts[co][:],
                                start=(co == 0),
                                stop=(co == 7),
                            )
                        yst = ystp.tile([128, TQ], F32, tag="yst")
                        nc.vector.tensor_tensor(
                            yst[:], ps[:], borep[:, TQ * fo2 : TQ * fo2 + TQ], ALU.add
                        )
                        nc.sync.dma_start(
                            out=y_out[128 * to : 128 * to + 128, TQ * fo2 : TQ * fo2 + TQ],
                            in_=yst[:],
                        )

    _split_multi_waits(nc)
    return nc


_NC_CACHE = None


def _get_nc():
    global _NC_CACHE
    if _NC_CACHE is None:
        _NC_CACHE = _build()
    return _NC_CACHE


def _bf(x):
    return np.ascontiguousarray(np.asarray(x, np.float32).astype(ml_dtypes.bfloat16))


def _prepare_in_maps(
    query, key_, value, Wq, bq, Wk, bk, Wv, bv, Wo, bo, bias_table, offset
):
    query = np.asarray(query, np.float32)
    key_ = np.asarray(key_, np.float32)
    value = np.asarray(value, np.float32)
    shared = {
        "wqT": _bf(np.asarray(Wq, np.float32).T),
        "wkT": _bf(np.asarray(Wk, np.float32).T),
        "wvT": _bf(np.asarray(Wv, np.float32).T),
        "woT": _bf(np.asarray(Wo, np.float32).T),
        "bq2": np.ascontiguousarray(np.asarray(bq, np.float32).reshape(8, 128).T),
        "bk2": np.ascontiguousarray(np.asarray(bk, np.float32).reshape(8, 128).T),
        "bv1": np.ascontiguousarray(np.asarray(bv, np.float32)),
        "bo1": np.ascontiguousarray(np.asarray(bo, np.float32)),
        "offs": np.ascontiguousarray(np.asarray(offset, np.float32)),
    }
    tab = np.asarray(bias_table, np.float32)  # [2T-1, H]
    pad = np.concatenate([tab[0:1], tab, tab[-1:]], axis=0)  # [2T+1, H]
    shared["rtabp"] = np.ascontiguousarray(pad[::-1].T)  # [H, 2T+1]

    in_maps = []
    for b in range(B):
        m = dict(shared)
        m["xqT"] = _bf(query[b].T)
        m["xkT"] = _bf(key_[b].T)
        m["xvT"] = _bf(value[b].T)
        in_maps.append(m)
    return in_maps


def kernel(**inputs):
    in_maps = _prepare_in_maps(
        inputs["query"], inputs["key_"], inputs["value"],
        inputs["Wq"], inputs["bq"], inputs["Wk"], inputs["bk"],
        inputs["Wv"], inputs["bv"], inputs["Wo"], inputs["bo"],
        inputs["bias_table"], inputs["offset"],
    )
    nc = _get_nc()
    res = run_bass_kernel_spmd(nc, in_maps, list(range(B)))
    out = np.stack([res.results[b]["y"] for b in range(B)], axis=0)
    return out.astype(np.float32)



# revision 17
# speedup vs baseline: 1.5167x; 1.1984x over previous
"""Trainium2 Bass kernel for CustomTemporalAttention.

B=8, T=1024, E=1024, H=16, D=64. Sharding: pure batch data-parallel across the
8 NeuronCores (core b computes batch element b end-to-end; weights and the tiny
bias table are replicated). No collectives.

Per-core math (torch Linear convention x @ W.T + b):
  qT = Wq @ query[b].T  (stored transposed: [E, T], channel-major)
  kT likewise; v in [T, E] layout augmented with a ones column per head.
  Per head h: S^T[tk, tq] = sum_d kT[d,tk] qT[d,tq]
  P^T = exp(0.125 * (S^T + 8*biasT)) via DVE add + ACT exp(scale=0.125)
  [num; den] = [v_h | 1].T @ P^T  accumulated over tk chunks (PSUM [65, 512])
  O^T_h = num / den ; y = O @ Wo.T + bo.

Matmul operands are bf16 (fp32 PSUM accumulate): full-chain numpy model gives
5.1e-3 max rel err vs the fp32 reference. bf16 restores fast-weight-load and
LDWEIGHTS/ MATMUL overlap that fp32/f32r modes forfeit.

Temporal bias: bias(q,k) = lerp(table[q - k + T-1]) with a global fractional
shift u = tanh(offset)/2.  blend[r] = a*tabp[r] + b*tabp[r+1] + c*tabp[r+2]
with a=relu(-u), b=1-|u|, c=relu(u) and tabp edge-padded — exact including the
clipped endpoints. Toeplitz tiles blend[C' - i + j] are materialized per head
as BSp[i, y] = rblend[y + i] (one overlapping-window DMA from DRAM scratch)
and read back with reversed free-dim slices (both patterns HW-verified).
"""

import sys

sys.path.insert(0, "/opt/trn_rl_repo")

import ml_dtypes
import numpy as np

import concourse.bass as bass
import concourse.mybir as mybir
import concourse.tile as tile
from concourse.bass_utils import run_bass_kernel_spmd

F32 = mybir.dt.float32
F32R = mybir.dt.float32r
BF16 = mybir.dt.bfloat16
AF = mybir.ActivationFunctionType
ALU = mybir.AluOpType

B, T, E, H = 8, 1024, 1024, 16
D = E // H  # 64
TQ = 512
W_BSP = 1920


def _split_multi_waits(nc):
    """This walrus build accepts at most one sync-wait per instruction; hoist
    extras onto same-engine NoOp carriers placed immediately before."""
    n = 0
    for f in nc.m.functions:
        for blk in f.blocks:
            out = []
            for inst in blk.instructions:
                si = inst.sync_info
                waits = list(si.on_wait) if si and si.on_wait else []
                if len(waits) > 1:
                    for w in waits[:-1]:
                        n += 1
                        nop = mybir.InstNoOp(name=f"{inst.name}-ws{n}", ins=[], outs=[])
                        nop.engine = inst.engine
                        nop.sync_info = mybir.SyncInfo(on_wait=[w], on_update=[])
                        out.append(nop)
                    inst.sync_info = mybir.SyncInfo(
                        on_wait=[waits[-1]], on_update=list(si.on_update or [])
                    )
                out.append(inst)
            blk.instructions = out
    return n


def _craft(ap, dims, offset=None):
    c = ap.copy()
    c.ap = ap.ap.__class__(dims)
    if offset is not None:
        c.offset = offset
    return c


def _build():
    nc = bass.Bass()

    xqT = nc.declare_dram_parameter("xqT", [128, 8, T], BF16, isOutput=False)
    xkT = nc.declare_dram_parameter("xkT", [128, 8, T], BF16, isOutput=False)
    xvT = nc.declare_dram_parameter("xvT", [128, 8, T], BF16, isOutput=False)
    wqT = nc.declare_dram_parameter("wqT", [128, 8, E], BF16, isOutput=False)
    wkT = nc.declare_dram_parameter("wkT", [128, 8, E], BF16, isOutput=False)
    wvT = nc.declare_dram_parameter("wvT", [128, 8, E], BF16, isOutput=False)
    woT = nc.declare_dram_parameter("woT", [128, 8, E], BF16, isOutput=False)
    bq2 = nc.declare_dram_parameter("bq2", [128, 8], F32, isOutput=False)
    bk2 = nc.declare_dram_parameter("bk2", [128, 8], F32, isOutput=False)
    bv1 = nc.declare_dram_parameter("bv1", [E], F32, isOutput=False)
    bo1 = nc.declare_dram_parameter("bo1", [E], F32, isOutput=False)
    rtabp = nc.declare_dram_parameter("rtabp", [H, 2 * T + 1], F32, isOutput=False)
    offs = nc.declare_dram_parameter("offs", [1], F32, isOutput=False)
    y_out = nc.declare_dram_parameter("y", [T, E], F32, isOutput=True)

    with tile.TileContext(nc) as tc:
        with (
            tc.tile_pool(name="persist", bufs=1) as persist,
            tc.tile_pool(name="small", bufs=1) as small,
            tc.tile_pool(name="dram", bufs=1, space="DRAM") as drp,
        ):
            qT = persist.tile([128, 8, T], BF16, tag="qT")
            kT = persist.tile([128, 8, T], BF16, tag="kT")
            vp = persist.tile([128, 8, H, D + 1], BF16, tag="vp")
            oT = persist.tile([128, 8, T], BF16, tag="oT")
            bvrep = persist.tile([128, E], F32, tag="bvrep")
            borep = persist.tile([128, E], F32, tag="borep")
            bqs = small.tile([128, 8], F32, tag="bqs")
            bks = small.tile([128, 8], F32, tag="bks")

            nc.sync.dma_start(out=bqs[:], in_=bq2[:])
            nc.sync.dma_start(out=bks[:], in_=bk2[:])

            # ---- phase 0: blended relative-position table ----
            p0ctx = tc.tile_pool(name="p0", bufs=1)
            p0 = p0ctx.__enter__()
            tab = p0.tile([H, 2 * T + 1], F32, tag="tab")
            nc.sync.dma_start(out=tab[:], in_=rtabp[:])
            off_sb = p0.tile([1, 1], F32, tag="off")
            nc.sync.dma_start(out=off_sb[:], in_=offs[None, :])
            th = p0.tile([1, 1], F32, tag="th")
            nc.scalar.activation(th[:], off_sb[:], AF.Tanh)
            w8 = p0.tile([1, 1], F32, tag="w8")
            nc.vector.tensor_scalar_mul(w8[:], th[:], 4.0)  # 8*u = 4*tanh
            abc = p0.tile([1, 3], F32, tag="abc")
            nc.vector.tensor_scalar(abc[:, 0:1], w8[:], -1.0, 0.0, ALU.mult, ALU.max)
            nc.vector.tensor_scalar(abc[:, 2:3], w8[:], 1.0, 0.0, ALU.mult, ALU.max)
            tsum = p0.tile([1, 1], F32, tag="tsum")
            nc.vector.tensor_tensor(tsum[:], abc[:, 0:1], abc[:, 2:3], ALU.add)
            nc.vector.tensor_scalar(abc[:, 1:2], tsum[:], -1.0, 8.0, ALU.mult, ALU.add)
            abc_dram = drp.tile([3], F32, tag="abc_dram")
            nc.gpsimd.dma_start(out=abc_dram[None, :], in_=abc[:])
            abc16 = p0.tile([H, 3], F32, tag="abc16")
            nc.gpsimd.dma_start(out=abc16[:], in_=_craft(abc_dram[:], [[0, H], [1, 3]], 0))

            nblend = 2 * T - 1
            rb = p0.tile([H, nblend], F32, tag="rb")
            rb_t = p0.tile([H, nblend], F32, tag="rb_t")
            nc.vector.tensor_scalar(rb[:], tab[:, 2 : 2 + nblend], abc16[:, 0:1], None, ALU.mult)
            nc.vector.tensor_scalar(rb_t[:], tab[:, 1 : 1 + nblend], abc16[:, 1:2], None, ALU.mult)
            nc.vector.tensor_tensor(rb[:], rb[:], rb_t[:], ALU.add)
            nc.vector.tensor_scalar(rb_t[:], tab[:, 0:nblend], abc16[:, 2:3], None, ALU.mult)
            nc.vector.tensor_tensor(rb[:], rb[:], rb_t[:], ALU.add)
            # erb = exp(bias) of the blended table (bf16): per-head exp(b)
            # slabs are then pure overlapping-window DMA loads, no ACT work
            erb = p0.tile([H, nblend], BF16, tag="erb")
            nc.scalar.activation(erb[:], rb[:], AF.Exp, scale=0.125)
            erb_dram = drp.tile([H, nblend], BF16, tag="erb_dram")
            nc.gpsimd.dma_start(out=erb_dram[:], in_=erb[:])
            p0ctx.__exit__(None, None, None)

            # ---- phase 1: projections ----
            # x and weights arrive as single whole-tensor DMAs in [128, 8, ·]
            # layout (host pre-transposed, 16KB/partition contiguous); the
            # matmuls slice them directly, so the Sync engine issues 7 big
            # DMAs instead of ~230 small ones.
            with (
                tc.tile_pool(name="xt", bufs=1) as xtp,
                tc.tile_pool(name="wt", bufs=1) as wtp,
                tc.tile_pool(name="pps", bufs=4, space="PSUM") as pps,
            ):
                xq3 = xtp.tile([128, 8, T], BF16, tag="xq3")
                nc.sync.dma_start(out=xq3[:], in_=xqT[:])
                wq3 = wtp.tile([128, 8, E], BF16, tag="wq3")
                nc.sync.dma_start(out=wq3[:], in_=wqT[:])
                xk3 = xtp.tile([128, 8, T], BF16, tag="xk3")
                nc.sync.dma_start(out=xk3[:], in_=xkT[:])
                wk3 = wtp.tile([128, 8, E], BF16, tag="wk3")
                nc.sync.dma_start(out=wk3[:], in_=wkT[:])
                xv3 = xtp.tile([128, 8, T], BF16, tag="xv3")
                nc.sync.dma_start(out=xv3[:], in_=xvT[:])
                wv3 = wtp.tile([128, 8, E], BF16, tag="wv3")
                nc.sync.dma_start(out=wv3[:], in_=wvT[:])
                nc.sync.dma_start(
                    out=bvrep[:], in_=_craft(bv1[:], [[0, 128], [1, E]], 0)
                )
                nc.sync.dma_start(
                    out=borep[:], in_=_craft(bo1[:], [[0, 128], [1, E]], 0)
                )
                wo3 = persist.tile([128, 8, E], BF16, tag="wo3")
                nc.sync.dma_start(out=wo3[:], in_=woT[:])

                for x3, w3, dst, bias_sb in (
                    (xq3, wq3, qT, bqs),
                    (xk3, wk3, kT, bks),
                ):
                    for fo in range(8):
                        ps = [pps.tile([128, TQ], F32, tag="pps", name=f"pp{fo}_{i}") for i in range(2)]
                        for eo in range(8):
                            for tq in range(2):
                                nc.tensor.matmul(
                                    ps[tq][:],
                                    w3[:, eo, 128 * fo : 128 * fo + 128],
                                    x3[:, eo, TQ * tq : TQ * tq + TQ],
                                    start=(eo == 0),
                                    stop=(eo == 7),
                                )
                        for tq in range(2):
                            nc.vector.tensor_scalar(
                                dst[:, fo, TQ * tq : TQ * tq + TQ],
                                ps[tq][:],
                                1.0,
                                bias_sb[:, fo : fo + 1],
                                ALU.mult,
                                ALU.add,
                            )

                for fv in range(2):
                    for to in range(8):
                        ps = pps.tile([128, TQ], F32, tag="pps")
                        for eo in range(8):
                            nc.tensor.matmul(
                                ps[:],
                                xv3[:, eo, 128 * to : 128 * to + 128],
                                wv3[:, eo, TQ * fv : TQ * fv + TQ],
                                start=(eo == 0),
                                stop=(eo == 7),
                            )
                        nc.vector.tensor_tensor(
                            vp[:, to, 8 * fv : 8 * fv + 8, 0:D],
                            ps[:].rearrange("p (h d) -> p h d", d=D),
                            bvrep[:, TQ * fv : TQ * fv + TQ].rearrange(
                                "p (h d) -> p h d", d=D
                            ),
                            ALU.add,
                        )
                nc.vector.memset(vp[:, :, :, D : D + 1], 1.0)

            # ---- phase 2: attention ----
            # exp(0.125*(S + 8b)) = exp(0.125*S) * exp(b): the Toeplitz bias is
            # applied multiplicatively with a per-head exp(b) slab (bf16, DVE
            # 4x mode) instead of an fp32 PSUM add, and PV matmuls are emitted
            # as a block after the S block so the PE stream never stalls on
            # the exp chain.
            #
            # softmax normalize is fully on-chip: DVE reciprocal of the den row
            # (PSUM partition 64) -> rank-1 f32r matmul ones[1,64] (x) rec
            # broadcasts it across 64 PSUM partitions -> one DVE multiply
            # (PSUM x PSUM -> SBUF bf16). No DRAM roundtrips.
            with (
                tc.tile_pool(name="eb", bufs=2) as ebp,
                tc.tile_pool(name="pt", bufs=4) as ptp,
                tc.tile_pool(name="pt0", bufs=3) as pt0p,
                tc.tile_pool(name="sm", bufs=4) as smp,
                tc.tile_pool(name="onum", bufs=2) as onp,
                tc.tile_pool(name="sps", bufs=2, space="PSUM") as sps,
                tc.tile_pool(name="ops", bufs=3, space="PSUM") as ops,
                tc.tile_pool(name="rps", bufs=1, space="PSUM") as rps,
            ):
                ones1f = smp.tile([1, D], F32, tag="ones1f")
                nc.vector.memset(ones1f[:], 1.0)
                ones1 = smp.tile([1, D], F32R, tag="ones1")
                with nc.allow_low_precision(reason="f32r == f32 bits"):
                    nc.vector.tensor_copy(out=ones1[:], in_=ones1f[:])
                ebs = {}
                pend = {}

                def emit_eb(hh):
                    eb_ = ebp.tile([128, W_BSP], BF16, tag="eb", name=f"eb{hh}")
                    nc.sync.dma_start(
                        out=eb_[:],
                        in_=_craft(erb_dram[:], [[1, 128], [1, W_BSP]], hh * nblend),
                    )
                    ebs[hh] = eb_

                def _norm_recip(hh):
                    # 1/den via ln -> rank-1 broadcast -> exp(-x): ln/exp share
                    # the ACT table set with the softmax exp (no table reload),
                    # and each op is ~0.65us vs 3.3us for a 1-lane DVE
                    # reciprocal over 512 elements.
                    st = pend[hh]
                    for tq in range(2):
                        lden = smp.tile([1, TQ], F32R, tag="lden", name=f"ld{hh}_{tq}")
                        with nc.allow_low_precision(reason="f32r == f32 bits"):
                            nc.scalar.activation(
                                lden[:], st["opsum"][tq][D : D + 1, :], AF.Ln
                            )
                        st["rec"].append(lden)

                def _norm_tail(hh, tq):
                    st = pend[hh]
                    po_, hp0_ = st["po"], st["hp0"]
                    ldb = rps.tile([D, TQ], F32, tag="recb", name=f"rb{hh}_{tq}")
                    nc.tensor.matmul(
                        ldb[:],
                        ones1[:],
                        st["rec"][tq][:],
                        start=True,
                        stop=True,
                    )
                    recs = smp.tile([D, TQ], F32, tag="recs", name=f"rs{hh}_{tq}")
                    nc.scalar.activation(recs[:], ldb[:], AF.Exp, scale=-1.0)
                    if hp0_ == 0:
                        nc.vector.tensor_tensor(
                            oT[0:64, po_, TQ * tq : TQ * tq + TQ],
                            st["opsum"][tq][0:D, :],
                            recs[:],
                            ALU.mult,
                        )
                    else:
                        onrm = onp.tile([64, TQ], BF16, tag="onrm", name=f"om{hh}_{tq}")
                        nc.vector.tensor_tensor(
                            onrm[:], st["opsum"][tq][0:D, :], recs[:], ALU.mult
                        )
                        nc.gpsimd.dma_start(
                            out=oT[64:128, po_, TQ * tq : TQ * tq + TQ], in_=onrm[:]
                        )

                emit_eb(0)
                for h in range(H):
                    hp0 = 64 * (h % 2)
                    po = h // 2
                    if h + 1 < H:
                        emit_eb(h + 1)
                    eb = ebs.pop(h)
                    if h - 1 in pend:
                        _norm_recip(h - 1)
                    opsum = [
                        ops.tile([D + 1, TQ], F32, tag="ops", name=f"op{h}_{i}")
                        for i in range(2)
                    ]
                    pts = {}

                    def emit_pv(cc):
                        pt_ = pts.pop(cc)
                        for tq in range(2):
                            nc.tensor.matmul(
                                opsum[tq][:],
                                vp[:, cc, h, :],
                                pt_[:, TQ * tq : TQ * tq + TQ],
                                start=(cc == 0),
                                stop=(cc == 7),
                            )

                    for c in range(8):
                        if h - 1 in pend and c in (1, 2):
                            _norm_tail(h - 1, c - 1)
                            if c == 2:
                                pend.pop(h - 1)
                        # both tq halves share one 2-bank PSUM tile so the exp
                        # and bias-multiply run as single [128,1024] ops
                        spsum = sps.tile([128, 2 * TQ], F32, tag="sps", name=f"sp{h}_{c}")
                        for tq in range(2):
                            nc.tensor.matmul(
                                spsum[:, TQ * tq : TQ * tq + TQ],
                                kT[hp0 : hp0 + 64, po, 128 * c : 128 * c + 128],
                                qT[hp0 : hp0 + 64, po, TQ * tq : TQ * tq + TQ],
                                start=True,
                                stop=True,
                            )
                        pt0 = pt0p.tile([128, 2 * TQ], BF16, tag="pt0")
                        nc.scalar.activation(pt0[:], spsum[:], AF.Exp, scale=0.125)
                        s0 = 1023 + 128 * c
                        pt = ptp.tile([128, 2 * TQ], BF16, tag="pt")
                        nc.vector.tensor_tensor(
                            pt[:],
                            pt0[:],
                            eb[:, s0 - (2 * TQ - 1) : s0 + 1][:, ::-1],
                            ALU.mult,
                        )
                        pts[c] = pt
                        if c >= 1:
                            emit_pv(c - 1)
                    emit_pv(7)
                    pend[h] = {"po": po, "hp0": hp0, "opsum": opsum, "rec": []}
                _norm_recip(H - 1)
                _norm_tail(H - 1, 0)
                _norm_tail(H - 1, 1)
                pend.pop(H - 1)

            # ---- phase 3: output projection ----
            with (
                tc.tile_pool(name="yst", bufs=4) as ystp,
                tc.tile_pool(name="pps3", bufs=4, space="PSUM") as pps3,
            ):
                for fo2 in range(2):
                    for to in range(8):
                        ps = pps3.tile([128, TQ], F32, tag="pps3")
                        for co in range(8):
                            nc.tensor.matmul(
                                ps[:],
                                oT[:, co, 128 * to : 128 * to + 128],
                                wo3[:, co, TQ * fo2 : TQ * fo2 + TQ],
                                start=(co == 0),
                                stop=(co == 7),
                            )
                        yst = ystp.tile([128, TQ], F32, tag="yst")
                        nc.vector.tensor_tensor(
                            yst[:], ps[:], borep[:, TQ * fo2 : TQ * fo2 + TQ], ALU.add
                        )
                        nc.sync.dma_start(
                            out=y_out[128 * to : 128 * to + 128, TQ * fo2 : TQ * fo2 + TQ],
                            in_=yst[:],
                        )

    _split_multi_waits(nc)
    return nc


_NC_CACHE = None


def _get_nc():
    global _NC_CACHE
    if _NC_CACHE is None:
        _NC_CACHE = _build()
    return _NC_CACHE


def _bf(x):
    return np.ascontiguousarray(np.asarray(x, np.float32).astype(ml_dtypes.bfloat16))


def _b8(xT):
    """[E, N] channel-major -> [128, 8, N] bf16 (partition, eo-block, col)."""
    xT = np.asarray(xT, np.float32)
    n = xT.shape[1]
    return np.ascontiguousarray(
        xT.reshape(8, 128, n).transpose(1, 0, 2).astype(ml_dtypes.bfloat16)
    )


def _prepare_in_maps(
    query, key_, value, Wq, bq, Wk, bk, Wv, bv, Wo, bo, bias_table, offset
):
    query = np.asarray(query, np.float32)
    key_ = np.asarray(key_, np.float32)
    value = np.asarray(value, np.float32)
    shared = {
        "wqT": _b8(np.asarray(Wq, np.float32).T),
        "wkT": _b8(np.asarray(Wk, np.float32).T),
        "wvT": _b8(np.asarray(Wv, np.float32).T),
        "woT": _b8(np.asarray(Wo, np.float32).T),
        "bq2": np.ascontiguousarray(np.asarray(bq, np.float32).reshape(8, 128).T),
        "bk2": np.ascontiguousarray(np.asarray(bk, np.float32).reshape(8, 128).T),
        "bv1": np.ascontiguousarray(np.asarray(bv, np.float32)),
        "bo1": np.ascontiguousarray(np.asarray(bo, np.float32)),
        "offs": np.ascontiguousarray(np.asarray(offset, np.float32)),
    }
    tab = np.asarray(bias_table, np.float32)  # [2T-1, H]
    pad = np.concatenate([tab[0:1], tab, tab[-1:]], axis=0)  # [2T+1, H]
    shared["rtabp"] = np.ascontiguousarray(pad[::-1].T)  # [H, 2T+1]

    in_maps = []
    for b in range(B):
        m = dict(shared)
        m["xqT"] = _b8(query[b].T)
        m["xkT"] = _b8(key_[b].T)
        m["xvT"] = _b8(value[b].T)
        in_maps.append(m)
    return in_maps


def kernel(**inputs):
    in_maps = _prepare_in_maps(
        inputs["query"], inputs["key_"], inputs["value"],
        inputs["Wq"], inputs["bq"], inputs["Wk"], inputs["bk"],
        inputs["Wv"], inputs["bv"], inputs["Wo"], inputs["bo"],
        inputs["bias_table"], inputs["offset"],
    )
    nc = _get_nc()
    res = run_bass_kernel_spmd(nc, in_maps, list(range(B)))
    out = np.stack([res.results[b]["y"] for b in range(B)], axis=0)
    return out.astype(np.float32)

